# revision 14
# baseline (speedup 1.0000x reference)
"""GPTQ int4 dequant + matmul kernel for Trainium2, column-parallel over 8 cores.

Computes out = x @ dequant(qweight, qzeros, scales) + bias where
  qweight: [OC//8, IC_total] int32 (nibbles packed along OC rows)
  qzeros:  [G, IC_total//8]  int32 (nibbles packed along IC cols)
  scales:  [G, IC_total]     float32
  x:       [N, OC]           float32
  bias:    [IC_total]        float32
Sharding: IC (out_features) split across 8 cores; x replicated.

Per-core kernel structure (v4 — prologue fully overlapped with main loop):
  1. zp unpack via int16-lane shifts (4 instrs, 2 nibbles per int16 lane)
     -> PE-transpose (int16) -> [IC, G] per-partition scalars zp and -zp*s.
  2. qweight per j-tile (128 IC cols): DMA -> PE-transpose (bit-exact,
     int32-as-f32) -> int16-lane nibble unpack (4 shift/mask ops, strided
     writes) -> dequant to bf16 reading raw int16 nibbles (engines convert
     int->float on read; exact): 8 groups on DVE ((v-zp)*s tensor_scalar),
     24 groups on ACT (Identity: s*v + (-zp*s)) -> xbar transpose into
     [OC-part, KT, chunk] weight tiles.
  3. Bias pre-broadcast to [128, IC] fp32 via a one-time K=1 fp32 matmul.
  4. Main loop as (token-tile, chunk) cells over 4 psum chunks
     [256, 384, 384, 352]: 32 matmuls accumulate one psum bank, DVE drains
     psum + adds fp32 bias, out DMA'd from the scalar queue (deferred past
     the next j-tile emission so ACT dequant is never head-of-line
     blocked).  Cell schedule: chunk-0-only cells run while later j-tiles
     dequantize; deferred chunks of early token tiles are revisited
     (x re-DMA'd) interleaved through the steady phase.
  All DMA transposes are issued from the single sync queue — concurrent
  xbar use from two queues corrupts data (observed on HW).
"""

import sys

if "/opt/trn_rl_repo" not in sys.path:
    sys.path.insert(0, "/opt/trn_rl_repo")

from contextlib import ExitStack

import numpy as np
import ml_dtypes

from concourse import bacc, bass, mybir, tile

P = 128
PACK = 8

f32 = mybir.dt.float32
bf16 = mybir.dt.bfloat16
i32 = mybir.dt.int32
i16 = mybir.dt.int16
Alu = mybir.AluOpType
ActFn = mybir.ActivationFunctionType

# Full problem dims (hardcoded per harness contract)
N_FULL = 4096
K_FULL = 4096  # OC / in_features (contraction)
IC_TOTAL = 11008
G_FULL = 32
N_CORES = 8
IC_SHARD = IC_TOTAL // N_CORES  # 1376

N_DVE_GROUPS = 8  # dequant groups on DVE; rest on ACT


def _jtiles(ic):
    """IC j-tiles of <=128, last may be ragged (must stay %16 for xbar)."""
    tiles = []
    off = 0
    while off < ic:
        w = min(P, ic - off)
        assert w % 16 == 0, f"ragged j-tile {w} not multiple of 16"
        tiles.append((off, w))
        off += w
    return tiles


def _chunks(ic):
    """PSUM chunk layout: small first chunk so matmuls start early."""
    if ic == 1376:
        return [(0, 256), (256, 384), (640, 384), (1024, 352)]
    chunks = []
    start = 0
    for off, w in _jtiles(ic):
        if off + w - start > 512:
            chunks.append((start, off - start))
            start = off
    chunks.append((start, ic - start))
    return chunks


def build(nc, n=N_FULL, k=K_FULL, ic=IC_SHARD, g=G_FULL, debug_dump=False):
    """Emit the per-core program. All cores run the same program (SPMD)."""
    assert k % P == 0 and n % P == 0 and k // g == P
    KT = k // P  # contraction tiles (each == one quant group)
    NT = n // P  # token tiles
    jts = _jtiles(ic)
    NJ = len(jts)
    chunks = _chunks(ic)
    NC = len(chunks)
    # map j-tile -> (chunk index, offset within chunk)
    jt_chunk = []
    for off, w in jts:
        for ci, (c0, cw) in enumerate(chunks):
            if c0 <= off < c0 + cw:
                jt_chunk.append((ci, off - c0))
                break

    q_d = nc.dram_tensor("qweight", [k // PACK, ic], i32, kind="ExternalInput")
    qz_d = nc.dram_tensor("qzeros", [g, ic // PACK], i32, kind="ExternalInput")
    s_d = nc.dram_tensor("scales", [g, ic], f32, kind="ExternalInput")
    x_d = nc.dram_tensor("x", [n, k], f32, kind="ExternalInput")
    b_d = nc.dram_tensor("bias", [ic], f32, kind="ExternalInput")
    id128_d = nc.dram_tensor("id128_f32", [P, P], f32, kind="ExternalInput")
    id32b_d = nc.dram_tensor("id32_bf16", [g, g], bf16, kind="ExternalInput")
    ones_d = nc.dram_tensor("ones_f32", [1, P], f32, kind="ExternalInput")
    out_d = nc.dram_tensor("out", [n, ic], f32, kind="ExternalOutput")
    if debug_dump:
        dbg_zp = nc.dram_tensor("dbg_zp", [P, NJ, g], f32, kind="ExternalOutput")
        dbg_sT = nc.dram_tensor("dbg_sT", [P, NJ, g], f32, kind="ExternalOutput")
        dbg_bias = nc.dram_tensor("dbg_bias", [P, ic], f32, kind="ExternalOutput")
        dbg_xT = nc.dram_tensor("dbg_xT", [P, k // P, P], bf16, kind="ExternalOutput")
        dbg_ws = [nc.dram_tensor(f"dbg_w{ci}", [P, k // P, cw], bf16,
                                 kind="ExternalOutput")
                  for ci, (c0, cw) in enumerate(chunks)]

    with tile.TileContext(nc) as tc, ExitStack() as ctx:
        const = ctx.enter_context(tc.tile_pool(name="const", bufs=1))
        wpool = ctx.enter_context(tc.tile_pool(name="w", bufs=1))
        qw4p = ctx.enter_context(tc.tile_pool(name="qw4", bufs=2))
        qwTp = ctx.enter_context(tc.tile_pool(name="qwT", bufs=2))
        nibp = ctx.enter_context(tc.tile_pool(name="nib", bufs=2))
        wtp = ctx.enter_context(tc.tile_pool(name="wt", bufs=2))
        xbp = ctx.enter_context(tc.tile_pool(name="xb", bufs=2))
        xTp = ctx.enter_context(tc.tile_pool(name="xT", bufs=3))
        opool = ctx.enter_context(tc.tile_pool(name="o", bufs=4))
        psum = ctx.enter_context(tc.tile_pool(name="psum", bufs=4, space="PSUM"))
        psum_t = ctx.enter_context(tc.tile_pool(name="psum_t", bufs=2, space="PSUM"))

        # ---- constants (sync queue)
        id128 = const.tile([P, P], f32)
        nc.sync.dma_start(out=id128[:], in_=id128_d[:])
        id32b = const.tile([g, g], bf16)
        nc.sync.dma_start(out=id32b[:], in_=id32b_d[:])
        ones = const.tile([1, P], f32)
        nc.sync.dma_start(out=ones[:], in_=ones_d[:])
        bias_sb = const.tile([1, ic], f32)
        nc.sync.dma_start(out=bias_sb[:], in_=b_d[None, :])
        qz_sb = const.tile([g, ic // PACK], i32)
        nc.sync.dma_start(out=qz_sb[:], in_=qz_d[:])
        s_sb = const.tile([g, ic], f32)
        nc.sync.dma_start(out=s_sb[:], in_=s_d[:])

        # ---- bias broadcast to [128, ic] fp32 via K=1 fp32 matmul
        bias_bc = const.tile([P, ic], f32)
        for ci, (c0, cw) in enumerate(chunks):
            pb = psum.tile([P, 512], f32, name="ps")
            nc.tensor.matmul(
                pb[:, :cw], lhsT=ones[:, :], rhs=bias_sb[:, c0 : c0 + cw],
                start=True, stop=True,
            )
            nc.vector.tensor_copy(bias_bc[:, c0 : c0 + cw], pb[:, :cw])

        # ---- zp unpack: qzeros [g, ic//8] i32 -> zp16 [g, ic] i16 (raw 0..15)
        #      int16 halves of each int32 hold nibbles 0-3 / 4-7; shift s
        #      extracts int16-elem j -> col 4j+s/4.
        zp16 = const.tile([g, ic], i16)
        qz16 = qz_sb.bitcast(i16)  # [g, ic//4]
        for s in (0, 4, 8, 12):
            nc.vector.tensor_scalar(
                out=zp16[:, s // 4 :: 4], in0=qz16[:], scalar1=s, scalar2=15,
                op0=Alu.logical_shift_right, op1=Alu.bitwise_and,
            )
        # |0x4300 -> bits of bf16(128+zp): keeps the PE transpose in normal
        # bf16 range (int16 Ldweights is rejected; bf16 denormals would be
        # flushed).  128 is subtracted after the f32 copy.
        nc.vector.tensor_scalar(
            out=zp16[:], in0=zp16[:], scalar1=0x4300, scalar2=None,
            op0=Alu.bitwise_or,
        )

        # ---- transpose zp/s to [IC-part, NJ, g]
        zp_pl = const.tile([P, NJ, g], f32)  # zp
        sT = const.tile([P, NJ, g], f32)     # s
        for ji, (off, w) in enumerate(jts):
            pz = psum_t.tile([P, g], bf16, name="pst")
            nc.tensor.transpose(
                pz[:w, :g], zp16.bitcast(bf16)[:, off : off + w], id32b[:]
            )
            nc.vector.tensor_copy(zp_pl[:w, ji, :], pz[:w, :g])
            ps_ = psum_t.tile([P, g], f32, name="pst")
            nc.tensor.transpose(ps_[:w, :g], s_sb[:, off : off + w], id128[:g, :g])
            nc.vector.tensor_copy(sT[:w, ji, :], ps_[:w, :g])
        nc.vector.tensor_scalar(out=zp_pl[:], in0=zp_pl[:], scalar1=128.0,
                                scalar2=None, op0=Alu.subtract)
        nzs = const.tile([P, NJ, g], f32)    # -zp*s (ACT dequant bias)
        nc.vector.tensor_tensor(out=nzs[:], in0=zp_pl[:], in1=sT[:], op=Alu.mult)
        nc.vector.tensor_scalar(out=nzs[:], in0=nzs[:], scalar1=-1.0,
                                scalar2=None, op0=Alu.mult)

        # ---- W chunks in [OC-part, KT, chunk-width] bf16
        wtiles = [wpool.tile([P, KT, cw], bf16, name=f"Wc{ci}")
                  for ci, (c0, cw) in enumerate(chunks)]

        RP = k // PACK  # packed qweight rows (512)
        assert RP % P == 0
        NRT = RP // P

        def emit_jt(ji):
            """Dequantize j-tile ji into its W chunk slice."""
            off, w = jts[ji]
            qw4 = qw4p.tile([P, NRT, P], i32, name="qw4")
            for rt in range(NRT):
                r0 = rt * P
                nc.gpsimd.dma_start(
                    out=qw4[:, rt, :w], in_=q_d[r0 : r0 + P, off : off + w]
                )
            # PE-transpose (bit-exact) -> qwT [w, RP packed words]
            qwT = qwTp.tile([P, RP], i32, name="qwT")
            for rt in range(NRT):
                r0 = rt * P
                pq = psum_t.tile([P, P], f32, name="pst")
                nc.tensor.transpose(
                    pq[:w, :P], qw4.bitcast(f32)[:, rt, :w], id128[:]
                )
                nc.vector.tensor_copy(qwT.bitcast(f32)[:w, r0 : r0 + P], pq[:w, :P])
            # int16-lane nibble unpack: int16 elem j, shift s -> col 4j+s/4
            nib = nibp.tile([P, k], i16, name="nib")
            qw16 = qwT.bitcast(i16)  # [P, k//4]
            for s in (0, 4, 8, 12):
                nc.vector.tensor_scalar(
                    out=nib[:w, s // 4 :: 4], in0=qw16[:w, :], scalar1=s,
                    scalar2=15, op0=Alu.logical_shift_right, op1=Alu.bitwise_and,
                )
            # dequant raw int16 nibbles -> bf16 (engines convert int->float)
            wt = wtp.tile([P, k], bf16, name="wt")
            for gi in range(g):
                o = wt[:w, gi * P : (gi + 1) * P]
                src = nib[:w, gi * P : (gi + 1) * P]
                if gi < N_DVE_GROUPS:
                    nc.vector.tensor_scalar(
                        out=o, in0=src,
                        scalar1=zp_pl[:w, ji, gi : gi + 1],
                        scalar2=sT[:w, ji, gi : gi + 1],
                        op0=Alu.subtract, op1=Alu.mult,
                    )
                else:
                    nc.scalar.activation(
                        out=o, in_=src, func=ActFn.Identity,
                        scale=sT[:w, ji, gi : gi + 1],
                        bias=nzs[:w, ji, gi : gi + 1],
                    )
            ci, coff = jt_chunk[ji]
            nc.sync.dma_start_transpose(
                out=wtiles[ci][:, :, coff : coff + w], in_=wt[:w, :]
            )

        # ---- x tile prep: gpsimd cast-load + sync-queue xbar transpose
        xcache = {}

        def emit_x(xkey):
            nt = xkey[0]
            xb = xbp.tile([P, k], bf16, name="xb")
            nc.gpsimd.dma_start(out=xb[:], in_=x_d[nt * P : (nt + 1) * P, :])
            xT = xTp.tile([P, KT, P], bf16, name="xT")
            nc.sync.dma_start_transpose(out=xT[:], in_=xb[:])
            xcache[xkey] = xT
            if debug_dump and xkey == (0, 0):
                nc.sync.dma_start(out=dbg_xT[:], in_=xT[:])

        pending_outs = []

        def emit_cell(nt, ci, xkey):
            c0, cw = chunks[ci]
            xT = xcache[xkey]
            ps = psum.tile([P, 512], f32, name="ps")
            for kt in range(KT):
                nc.tensor.matmul(
                    ps[:, :cw],
                    lhsT=xT[:, kt, :],
                    rhs=wtiles[ci][:, kt, :],
                    start=(kt == 0),
                    stop=(kt == KT - 1),
                )
            osb = opool.tile([P, 512], f32, name="osb")
            nc.vector.tensor_tensor(
                out=osb[:, :cw], in0=ps[:, :cw], in1=bias_bc[:, c0 : c0 + cw],
                op=Alu.add,
            )
            pending_outs.append(
                (osb[:, :cw], out_d[nt * P : (nt + 1) * P, c0 : c0 + cw])
            )

        def flush_outs():
            while pending_outs:
                src, dst = pending_outs.pop(0)
                nc.scalar.dma_start(out=dst, in_=src)

        # ---- schedule ----------------------------------------------------
        # ramp: nts 0..13 chunk-0 only, j-tiles 2..10 interleaved
        # partial: nts 14..19 chunks 0-2 (chunk 3 deferred)
        # steady: fulls nts 20..31 (all 4 chunks) alternating with revisit
        #         groups; single-cell revisits first, 3-cell revisits last.
        assert NT == 32 and NC == 4
        sched = []
        ramp_nts = list(range(14))
        jq = list(range(2, NJ))
        for i, nt in enumerate(ramp_nts):
            sched.append((nt, 0, 0))
            if i < len(jq):
                sched.append(("J", jq[i]))
        sched += [("J", j) for j in jq[len(ramp_nts):]]
        partial_nts = list(range(14, 20))
        for nt in partial_nts:
            for ci in range(3):
                sched.append((nt, ci, 0))
        fulls = [[(nt, ci, 0) for ci in range(NC)] for nt in range(20, NT)]
        groups = [[(nt, 3, 1)] for nt in partial_nts]          # single-cell
        groups += [[(nt, ci, 1) for ci in (1, 2, 3)] for nt in ramp_nts]
        gi_ = 0
        for f in fulls:
            sched += f
            if gi_ < len(groups):
                sched += groups[gi_]
                gi_ += 1
        for grp in groups[gi_:]:
            sched += grp

        # sanity: every (nt, ci) exactly once
        seen = set()
        for it in sched:
            if it[0] == "J":
                continue
            assert (it[0], it[1]) not in seen
            seen.add((it[0], it[1]))
        assert len(seen) == NT * NC, len(seen)

        # x-prefetch bookkeeping: xkey = (nt, pass)
        xorder = []
        for it in sched:
            if it[0] == "J":
                continue
            xk = (it[0], it[2])
            if xk not in xorder:
                xorder.append(xk)
        xpos = {xk: i for i, xk in enumerate(xorder)}
        nxt = 0

        def prefetch(upto):
            nonlocal nxt
            while nxt < min(upto, len(xorder)):
                emit_x(xorder[nxt])
                nxt += 1

        # emit chunk-0 j-tiles up front, then walk the schedule
        emit_jt(0)
        emit_jt(1)
        prefetch(2)
        for it in sched:
            if it[0] == "J":
                emit_jt(it[1])
                flush_outs()
                continue
            nt, ci, pss = it
            xk = (nt, pss)
            prefetch(xpos[xk] + 1)
            emit_cell(nt, ci, xk)
            prefetch(xpos[xk] + 3)  # keep 2 keys ahead (xT ring has 3 bufs)
        flush_outs()

        if debug_dump:
            nc.sync.dma_start(out=dbg_zp[:], in_=zp_pl[:])
            nc.sync.dma_start(out=dbg_sT[:], in_=sT[:])
            nc.sync.dma_start(out=dbg_bias[:], in_=bias_bc[:])
            for ci in range(NC):
                nc.sync.dma_start(out=dbg_ws[ci][:], in_=wtiles[ci][:])
    return nc


def make_const_inputs(g=G_FULL):
    return {
        "id128_f32": np.eye(P, dtype=np.float32),
        "id32_bf16": np.eye(g, dtype=ml_dtypes.bfloat16),
        "ones_f32": np.ones((1, P), dtype=np.float32),
    }


def kernel(input, qweight, qzeros, scales, bias):
    """Full-problem entry point: shard, run on 8 cores, gather."""
    from concourse.bass_utils import run_bass_kernel_spmd

    nc = bacc.Bacc("TRN2", target_bir_lowering=False, debug=False)
    build(nc)
    nc.compile()

    consts = make_const_inputs()
    x = np.ascontiguousarray(input, dtype=np.float32)
    in_maps = []
    for c in range(N_CORES):
        j0, j1 = c * IC_SHARD, (c + 1) * IC_SHARD
        in_maps.append(
            {
                "qweight": np.ascontiguousarray(qweight[:, j0:j1]),
                "qzeros": np.ascontiguousarray(
                    qzeros[:, c * (IC_SHARD // PACK) : (c + 1) * (IC_SHARD // PACK)]
                ),
                "scales": np.ascontiguousarray(scales[:, j0:j1]),
                "x": x,
                "bias": np.ascontiguousarray(bias[j0:j1]),
                **consts,
            }
        )
    res = run_bass_kernel_spmd(nc, in_maps, list(range(N_CORES)))
    outs = [np.asarray(res.results[c]["out"], dtype=np.float32) for c in range(N_CORES)]
    return np.concatenate(outs, axis=1)


# revision 16
# speedup vs baseline: 1.0150x; 1.0150x over previous
"""GPTQ int4 dequant + matmul kernel for Trainium2, column-parallel over 8 cores.

Computes out = x @ dequant(qweight, qzeros, scales) + bias where
  qweight: [OC//8, IC_total] int32 (nibbles packed along OC rows)
  qzeros:  [G, IC_total//8]  int32 (nibbles packed along IC cols)
  scales:  [G, IC_total]     float32
  x:       [N, OC]           float32
  bias:    [IC_total]        float32
Sharding: IC (out_features) split across 8 cores; x replicated.

Per-core kernel structure (v4 — prologue fully overlapped with main loop):
  1. zp unpack via int16-lane shifts (4 instrs, 2 nibbles per int16 lane)
     -> PE-transpose (int16) -> [IC, G] per-partition scalars zp and -zp*s.
  2. qweight per j-tile (128 IC cols): DMA -> PE-transpose (bit-exact,
     int32-as-f32) -> int16-lane nibble unpack (4 shift/mask ops, strided
     writes) -> dequant to bf16 reading raw int16 nibbles (engines convert
     int->float on read; exact): 8 groups on DVE ((v-zp)*s tensor_scalar),
     24 groups on ACT (Identity: s*v + (-zp*s)) -> xbar transpose into
     [OC-part, KT, chunk] weight tiles.
  3. Bias pre-broadcast to [128, IC] fp32 via a one-time K=1 fp32 matmul.
  4. Main loop as (token-tile, chunk) cells over 4 psum chunks
     [256, 384, 384, 352]: 32 matmuls accumulate one psum bank, DVE drains
     psum + adds fp32 bias, out DMA'd from the scalar queue (deferred past
     the next j-tile emission so ACT dequant is never head-of-line
     blocked).  Cell schedule: chunk-0-only cells run while later j-tiles
     dequantize; deferred chunks of early token tiles are revisited
     (x re-DMA'd) interleaved through the steady phase.
  All DMA transposes are issued from the single sync queue — concurrent
  xbar use from two queues corrupts data (observed on HW).
"""

import sys

if "/opt/trn_rl_repo" not in sys.path:
    sys.path.insert(0, "/opt/trn_rl_repo")

from contextlib import ExitStack

import numpy as np
import ml_dtypes

from concourse import bacc, bass, mybir, tile

P = 128
PACK = 8

f32 = mybir.dt.float32
bf16 = mybir.dt.bfloat16
i32 = mybir.dt.int32
i16 = mybir.dt.int16
Alu = mybir.AluOpType
ActFn = mybir.ActivationFunctionType

# Full problem dims (hardcoded per harness contract)
N_FULL = 4096
K_FULL = 4096  # OC / in_features (contraction)
IC_TOTAL = 11008
G_FULL = 32
N_CORES = 8
IC_SHARD = IC_TOTAL // N_CORES  # 1376

N_DVE_GROUPS = 12  # dequant groups on DVE; rest on ACT


def _jtiles(ic):
    """IC j-tiles of <=128, last may be ragged (must stay %16 for xbar)."""
    tiles = []
    off = 0
    while off < ic:
        w = min(P, ic - off)
        assert w % 16 == 0, f"ragged j-tile {w} not multiple of 16"
        tiles.append((off, w))
        off += w
    return tiles


def _chunks(ic):
    """PSUM chunk layout: small first chunk so matmuls start early."""
    if ic == 1376:
        return [(0, 256), (256, 384), (640, 384), (1024, 352)]
    chunks = []
    start = 0
    for off, w in _jtiles(ic):
        if off + w - start > 512:
            chunks.append((start, off - start))
            start = off
    chunks.append((start, ic - start))
    return chunks


def build(nc, n=N_FULL, k=K_FULL, ic=IC_SHARD, g=G_FULL, debug_dump=False):
    """Emit the per-core program. All cores run the same program (SPMD)."""
    assert k % P == 0 and n % P == 0 and k // g == P
    KT = k // P  # contraction tiles (each == one quant group)
    NT = n // P  # token tiles
    jts = _jtiles(ic)
    NJ = len(jts)
    chunks = _chunks(ic)
    NC = len(chunks)
    # map j-tile -> (chunk index, offset within chunk)
    jt_chunk = []
    for off, w in jts:
        for ci, (c0, cw) in enumerate(chunks):
            if c0 <= off < c0 + cw:
                jt_chunk.append((ci, off - c0))
                break

    q_d = nc.dram_tensor("qweight", [k // PACK, ic], i32, kind="ExternalInput")
    qz_d = nc.dram_tensor("qzeros", [g, ic // PACK], i32, kind="ExternalInput")
    s_d = nc.dram_tensor("scales", [g, ic], f32, kind="ExternalInput")
    x_d = nc.dram_tensor("x", [n, k], f32, kind="ExternalInput")
    b_d = nc.dram_tensor("bias", [ic], f32, kind="ExternalInput")
    id128_d = nc.dram_tensor("id128_f32", [P, P], f32, kind="ExternalInput")
    id32b_d = nc.dram_tensor("id32_bf16", [g, g], bf16, kind="ExternalInput")
    ones_d = nc.dram_tensor("ones_f32", [1, P], f32, kind="ExternalInput")
    out_d = nc.dram_tensor("out", [n, ic], f32, kind="ExternalOutput")
    if debug_dump:
        dbg_zp = nc.dram_tensor("dbg_zp", [P, NJ, g], f32, kind="ExternalOutput")
        dbg_sT = nc.dram_tensor("dbg_sT", [P, NJ, g], f32, kind="ExternalOutput")
        dbg_bias = nc.dram_tensor("dbg_bias", [P, ic], f32, kind="ExternalOutput")
        dbg_xT = nc.dram_tensor("dbg_xT", [P, k // P, P], bf16, kind="ExternalOutput")
        dbg_ws = [nc.dram_tensor(f"dbg_w{ci}", [P, k // P, cw], bf16,
                                 kind="ExternalOutput")
                  for ci, (c0, cw) in enumerate(chunks)]

    with tile.TileContext(nc) as tc, ExitStack() as ctx:
        const = ctx.enter_context(tc.tile_pool(name="const", bufs=1))
        wpool = ctx.enter_context(tc.tile_pool(name="w", bufs=1))
        qw4p = ctx.enter_context(tc.tile_pool(name="qw4", bufs=2))
        qwTp = ctx.enter_context(tc.tile_pool(name="qwT", bufs=2))
        nibp = ctx.enter_context(tc.tile_pool(name="nib", bufs=2))
        wtp = ctx.enter_context(tc.tile_pool(name="wt", bufs=2))
        xbp = ctx.enter_context(tc.tile_pool(name="xb", bufs=2))
        xTp = ctx.enter_context(tc.tile_pool(name="xT", bufs=3))
        opool = ctx.enter_context(tc.tile_pool(name="o", bufs=4))
        psum = ctx.enter_context(tc.tile_pool(name="psum", bufs=4, space="PSUM"))
        psum_t = ctx.enter_context(tc.tile_pool(name="psum_t", bufs=2, space="PSUM"))

        # ---- constants (sync queue)
        id128 = const.tile([P, P], f32)
        nc.sync.dma_start(out=id128[:], in_=id128_d[:])
        id32b = const.tile([g, g], bf16)
        nc.sync.dma_start(out=id32b[:], in_=id32b_d[:])
        ones = const.tile([1, P], f32)
        nc.sync.dma_start(out=ones[:], in_=ones_d[:])
        bias_sb = const.tile([1, ic], f32)
        nc.sync.dma_start(out=bias_sb[:], in_=b_d[None, :])
        qz_sb = const.tile([g, ic // PACK], i32)
        nc.sync.dma_start(out=qz_sb[:], in_=qz_d[:])
        s_sb = const.tile([g, ic], f32)
        nc.sync.dma_start(out=s_sb[:], in_=s_d[:])

        # ---- bias broadcast to [128, ic] fp32 via K=1 fp32 matmul
        bias_bc = const.tile([P, ic], f32)
        for ci, (c0, cw) in enumerate(chunks):
            pb = psum.tile([P, 512], f32, name="ps")
            nc.tensor.matmul(
                pb[:, :cw], lhsT=ones[:, :], rhs=bias_sb[:, c0 : c0 + cw],
                start=True, stop=True,
            )
            nc.vector.tensor_copy(bias_bc[:, c0 : c0 + cw], pb[:, :cw])

        # ---- zp unpack: qzeros [g, ic//8] i32 -> zp16 [g, ic] i16 (raw 0..15)
        #      int16 halves of each int32 hold nibbles 0-3 / 4-7; shift s
        #      extracts int16-elem j -> col 4j+s/4.
        zp16 = const.tile([g, ic], i16)
        qz16 = qz_sb.bitcast(i16)  # [g, ic//4]
        for s in (0, 4, 8, 12):
            nc.vector.tensor_scalar(
                out=zp16[:, s // 4 :: 4], in0=qz16[:], scalar1=s, scalar2=15,
                op0=Alu.logical_shift_right, op1=Alu.bitwise_and,
            )
        # |0x4300 -> bits of bf16(128+zp): keeps the PE transpose in normal
        # bf16 range (int16 Ldweights is rejected; bf16 denormals would be
        # flushed).  128 is subtracted after the f32 copy.
        nc.vector.tensor_scalar(
            out=zp16[:], in0=zp16[:], scalar1=0x4300, scalar2=None,
            op0=Alu.bitwise_or,
        )

        # ---- transpose zp/s to [IC-part, NJ, g]
        zp_pl = const.tile([P, NJ, g], f32)  # zp
        sT = const.tile([P, NJ, g], f32)     # s
        for ji, (off, w) in enumerate(jts):
            pz = psum_t.tile([P, g], bf16, name="pst")
            nc.tensor.transpose(
                pz[:w, :g], zp16.bitcast(bf16)[:, off : off + w], id32b[:]
            )
            nc.vector.tensor_copy(zp_pl[:w, ji, :], pz[:w, :g])
            ps_ = psum_t.tile([P, g], f32, name="pst")
            nc.tensor.transpose(ps_[:w, :g], s_sb[:, off : off + w], id128[:g, :g])
            nc.vector.tensor_copy(sT[:w, ji, :], ps_[:w, :g])
        nc.vector.tensor_scalar(out=zp_pl[:], in0=zp_pl[:], scalar1=128.0,
                                scalar2=None, op0=Alu.subtract)
        nzs = const.tile([P, NJ, g], f32)    # -zp*s (ACT dequant bias)
        nc.vector.tensor_tensor(out=nzs[:], in0=zp_pl[:], in1=sT[:], op=Alu.mult)
        nc.vector.tensor_scalar(out=nzs[:], in0=nzs[:], scalar1=-1.0,
                                scalar2=None, op0=Alu.mult)

        # ---- W chunks in [OC-part, KT, chunk-width] bf16
        wtiles = [wpool.tile([P, KT, cw], bf16, name=f"Wc{ci}")
                  for ci, (c0, cw) in enumerate(chunks)]

        RP = k // PACK  # packed qweight rows (512)
        assert RP % P == 0
        NRT = RP // P

        # W xbar transposes are deferred by one j-tile: enqueueing them on
        # the sync queue while their dequant is still running would
        # head-of-line block the xT transposes behind them.
        pending_xbars = []

        def flush_xbars():
            while pending_xbars:
                ci, coff, w, wt = pending_xbars.pop(0)
                nc.sync.dma_start_transpose(
                    out=wtiles[ci][:, :, coff : coff + w], in_=wt[:w, :]
                )

        def emit_jt(ji):
            """Dequantize j-tile ji into its W chunk slice."""
            off, w = jts[ji]
            qw4 = qw4p.tile([P, NRT, P], i32, name="qw4")
            for rt in range(NRT):
                r0 = rt * P
                nc.gpsimd.dma_start(
                    out=qw4[:, rt, :w], in_=q_d[r0 : r0 + P, off : off + w]
                )
            # PE-transpose (bit-exact) -> qwT [w, RP packed words]
            qwT = qwTp.tile([P, RP], i32, name="qwT")
            for rt in range(NRT):
                r0 = rt * P
                pq = psum_t.tile([P, P], f32, name="pst")
                nc.tensor.transpose(
                    pq[:w, :P], qw4.bitcast(f32)[:, rt, :w], id128[:]
                )
                nc.vector.tensor_copy(qwT.bitcast(f32)[:w, r0 : r0 + P], pq[:w, :P])
            # int16-lane nibble unpack: int16 elem j, shift s -> col 4j+s/4
            nib = nibp.tile([P, k], i16, name="nib")
            qw16 = qwT.bitcast(i16)  # [P, k//4]
            for s in (0, 4, 8, 12):
                nc.vector.tensor_scalar(
                    out=nib[:w, s // 4 :: 4], in0=qw16[:w, :], scalar1=s,
                    scalar2=15, op0=Alu.logical_shift_right, op1=Alu.bitwise_and,
                )
            # dequant raw int16 nibbles -> bf16 (engines convert int->float)
            wt = wtp.tile([P, k], bf16, name="wt")
            for gi in range(g):
                o = wt[:w, gi * P : (gi + 1) * P]
                src = nib[:w, gi * P : (gi + 1) * P]
                if gi < N_DVE_GROUPS:
                    nc.vector.tensor_scalar(
                        out=o, in0=src,
                        scalar1=zp_pl[:w, ji, gi : gi + 1],
                        scalar2=sT[:w, ji, gi : gi + 1],
                        op0=Alu.subtract, op1=Alu.mult,
                    )
                else:
                    nc.scalar.activation(
                        out=o, in_=src, func=ActFn.Identity,
                        scale=sT[:w, ji, gi : gi + 1],
                        bias=nzs[:w, ji, gi : gi + 1],
                    )
            ci, coff = jt_chunk[ji]
            pending_xbars.append((ci, coff, w, wt))

        # ---- x tile prep: gpsimd cast-load + sync-queue xbar transpose
        xcache = {}

        def emit_x(xkey):
            nt = xkey[0]
            xb = xbp.tile([P, k], bf16, name="xb")
            nc.gpsimd.dma_start(out=xb[:], in_=x_d[nt * P : (nt + 1) * P, :])
            xT = xTp.tile([P, KT, P], bf16, name="xT")
            nc.sync.dma_start_transpose(out=xT[:], in_=xb[:])
            xcache[xkey] = xT
            if debug_dump and xkey == (0, 0):
                nc.sync.dma_start(out=dbg_xT[:], in_=xT[:])

        pending_outs = []

        def emit_cell(nt, ci, xkey):
            c0, cw = chunks[ci]
            xT = xcache[xkey]
            ps = psum.tile([P, 512], f32, name="ps")
            for kt in range(KT):
                nc.tensor.matmul(
                    ps[:, :cw],
                    lhsT=xT[:, kt, :],
                    rhs=wtiles[ci][:, kt, :],
                    start=(kt == 0),
                    stop=(kt == KT - 1),
                )
            osb = opool.tile([P, 512], f32, name="osb")
            nc.vector.tensor_tensor(
                out=osb[:, :cw], in0=ps[:, :cw], in1=bias_bc[:, c0 : c0 + cw],
                op=Alu.add,
            )
            pending_outs.append(
                (osb[:, :cw], out_d[nt * P : (nt + 1) * P, c0 : c0 + cw])
            )

        def flush_outs():
            while pending_outs:
                src, dst = pending_outs.pop(0)
                nc.scalar.dma_start(out=dst, in_=src)

        # ---- schedule ----------------------------------------------------
        # ramp: nts 0..13 chunk-0 only, j-tiles 2..10 interleaved
        # partial: nts 14..19 chunks 0-2 (chunk 3 deferred)
        # steady: fulls nts 20..31 (all 4 chunks) alternating with revisit
        #         groups; single-cell revisits first, 3-cell revisits last.
        assert NT == 32 and NC == 4
        sched = []
        ramp_nts = list(range(14))
        jq = list(range(2, NJ))
        for i, nt in enumerate(ramp_nts):
            sched.append((nt, 0, 0))
            if i < len(jq):
                sched.append(("J", jq[i]))
        sched += [("J", j) for j in jq[len(ramp_nts):]]
        partial_nts = list(range(14, 20))
        for nt in partial_nts:
            for ci in range(3):
                sched.append((nt, ci, 0))
        fulls = [[(nt, ci, 0) for ci in range(NC)] for nt in range(20, NT)]
        groups = [[(nt, 3, 1)] for nt in partial_nts]          # single-cell
        groups += [[(nt, ci, 1) for ci in (1, 2, 3)] for nt in ramp_nts]
        gi_ = 0
        for f in fulls:
            sched += f
            if gi_ < len(groups):
                sched += groups[gi_]
                gi_ += 1
        for grp in groups[gi_:]:
            sched += grp

        # sanity: every (nt, ci) exactly once
        seen = set()
        for it in sched:
            if it[0] == "J":
                continue
            assert (it[0], it[1]) not in seen
            seen.add((it[0], it[1]))
        assert len(seen) == NT * NC, len(seen)

        # x-prefetch bookkeeping: xkey = (nt, pass)
        xorder = []
        for it in sched:
            if it[0] == "J":
                continue
            xk = (it[0], it[2])
            if xk not in xorder:
                xorder.append(xk)
        xpos = {xk: i for i, xk in enumerate(xorder)}
        nxt = 0

        def prefetch(upto):
            nonlocal nxt
            while nxt < min(upto, len(xorder)):
                emit_x(xorder[nxt])
                nxt += 1

        # emit chunk-0 j-tiles up front, then walk the schedule
        emit_jt(0)
        flush_xbars()  # jt0 xbar ahead of the first xT prefetches
        emit_jt(1)
        prefetch(2)
        js_left = sum(1 for it in sched if it[0] == "J")
        for it in sched:
            if it[0] == "J":
                flush_xbars()
                emit_jt(it[1])
                flush_outs()
                js_left -= 1
                continue
            nt, ci, pss = it
            if (js_left == 0 and pending_xbars) or any(
                px[0] == ci for px in pending_xbars
            ):
                flush_xbars()
            xk = (nt, pss)
            prefetch(xpos[xk] + 1)
            emit_cell(nt, ci, xk)
            prefetch(xpos[xk] + 3)  # keep 2 keys ahead (xT ring has 3 bufs)
        flush_outs()

        if debug_dump:
            nc.sync.dma_start(out=dbg_zp[:], in_=zp_pl[:])
            nc.sync.dma_start(out=dbg_sT[:], in_=sT[:])
            nc.sync.dma_start(out=dbg_bias[:], in_=bias_bc[:])
            for ci in range(NC):
                nc.sync.dma_start(out=dbg_ws[ci][:], in_=wtiles[ci][:])
    return nc


def make_const_inputs(g=G_FULL):
    return {
        "id128_f32": np.eye(P, dtype=np.float32),
        "id32_bf16": np.eye(g, dtype=ml_dtypes.bfloat16),
        "ones_f32": np.ones((1, P), dtype=np.float32),
    }


def kernel(input, qweight, qzeros, scales, bias):
    """Full-problem entry point: shard, run on 8 cores, gather."""
    from concourse.bass_utils import run_bass_kernel_spmd

    nc = bacc.Bacc("TRN2", target_bir_lowering=False, debug=False)
    build(nc)
    nc.compile()

    consts = make_const_inputs()
    x = np.ascontiguousarray(input, dtype=np.float32)
    in_maps = []
    for c in range(N_CORES):
        j0, j1 = c * IC_SHARD, (c + 1) * IC_SHARD
        in_maps.append(
            {
                "qweight": np.ascontiguousarray(qweight[:, j0:j1]),
                "qzeros": np.ascontiguousarray(
                    qzeros[:, c * (IC_SHARD // PACK) : (c + 1) * (IC_SHARD // PACK)]
                ),
                "scales": np.ascontiguousarray(scales[:, j0:j1]),
                "x": x,
                "bias": np.ascontiguousarray(bias[j0:j1]),
                **consts,
            }
        )
    res = run_bass_kernel_spmd(nc, in_maps, list(range(N_CORES)))
    outs = [np.asarray(res.results[c]["out"], dtype=np.float32) for c in range(N_CORES)]
    return np.concatenate(outs, axis=1)
